# revision 41
# baseline (speedup 1.0000x reference)
"""Trainium2 Bass kernel for nn_DenoisingPotential — score-space formulation.

Reference iterates x <- x + alpha * grad_phi(x) 10 times where
  grad_phi(x) = -sum_k softmax_k(c_k - 0.5 |x-mu_k|_P^2) P (x-mu_k)
with P_k = A_k^T A_k.  For the shipped inputs P_k == I, so with
pm_k = P mu_k, M = I - alpha P = m*I (m = 0.9):

  r_t = pm @ x_t          (K=32 scores per sample)
  w_t = softmax(r_t + bias),  bias_k = c_k - 0.5 mu_k.pm_k
  x_{t+1} = m x_t + alpha pm^T w_t
  r_{t+1} = m r_t + alpha G w_t,   G = pm pm^T   (32x32!)

The whole 10-iteration dynamics lives in 32-dim score space; x is
touched only at entry (r_0 = pm x_0) and exit:
  x_10 = m^10 x_0 + pm^T u,   u = sum_t c_t w_t,  c_t = alpha m^(9-t).

With rho_t = r_t / m^t the score update is a pure accumulation
  rho_{t+1} = rho_t + (alpha/m^(t+1)) G w_t
kept in PSUM across all iterations (matmul start=False accumulation with
skip_group_check so interleaved ACT reads are legal), and
E_t = exp(m^t rho_t + bias) via the ACT scale argument.  c_t is folded
into the per-iteration Z weights (wZ_t = (1/c_t) * block-ones), so the
reciprocal directly yields c_t/Z and V_t = E*rz = c_t w_t; then
rho += Gs V_t with the constant Gs = m^-10 G, and u += V_t (plain
accumulation: PE identity-matmul into PSUM for chunks 0-1, Pool
tensor_add in SBUF for chunks 2-3).

All per-chunk state lives in per-chunk TILES (not slices of one big
tile): the Tile dependency tracker collapses access patterns to byte
intervals, so distinct column slices of one tile false-share and
serialize the pipeline.

Layout per core (8192 samples): sample PAIRS are interleaved on
partitions so every DMA descriptor is 512B (full DMA bus rate):
  x_nat[p, w, (e j)] = x[2*(128w+p)+e, j]        8 tiles (128, 4, 128)
  xT[(h j), q] = x[2q+h, j]  bf16                8 tiles (128, 512)
Score packing (128, 512) x 4 chunks: chunk c col c' rows 32g:32(g+1) =
scores of sample pair q = 2048*(g//2) + 512c + c', parity g%2.
Output is computed in NATURAL layout (no output transposes):
out[p, (e j)] = sum_k u[k, q] pm[k, j] via u-slices as the stationary
operand, with the m^10 * x0 term folded in as a second accumulating
matmul (m10-scaled identity applied to the bf16 x^T slices), then one
psum->sbuf copy per tile and a 512B-descriptor store.
"""

import os
import numpy as np

B = 65536
D = 64
K = 32
N_ITER = 10
N_CORES = 8
BC = B // N_CORES  # 8192 samples per core
NQ = BC // 2       # 4096 pairs (xT columns)
QB = BC // 4       # 2048 score-packing columns
CH = 512           # chunk free size
NW = NQ // 128     # 32 natural tiles of 128 pairs

_MODULE_CACHE = {}


def _build_module(m, alpha, n_ups=2, mult_dve=(), skew=(0, 1, 1, 1)):
    """m: scalar of M = m*I.
    n_ups: chunks (from 0) whose u accumulates in PSUM via PE; the rest
    accumulate in SBUF via Pool stt.
    mult_dve: chunks whose V=E*rz multiply runs on DVE instead of Pool."""
    import concourse.bacc as bacc
    import concourse.tile as tile
    from concourse import mybir
    from contextlib import ExitStack

    f32 = mybir.dt.float32
    bf16 = mybir.dt.bfloat16
    Exp = mybir.ActivationFunctionType.Exp
    Alu = mybir.AluOpType

    m64 = np.float64(m)

    nc = bacc.Bacc()

    x_in = nc.dram_tensor("x", [BC, D], f32, kind="ExternalInput")
    cbfA_in = nc.dram_tensor("cbfA", [128, 64 + N_ITER * 128], bf16,
                             kind="ExternalInput")
    cbfB_in = nc.dram_tensor("cbfB", [128, 512], bf16, kind="ExternalInput")
    cf32_in = nc.dram_tensor("cf32", [128, 129], f32, kind="ExternalInput")
    out = nc.dram_tensor("out", [BC, D], f32, kind="ExternalOutput")

    # DRAM views with pair-interleaved natural tiles (512B descriptors)
    xr = x_in.rearrange("(w p e) j -> p w (e j)", p=128, e=2)    # (128,32,128)
    outr = out.rearrange("(w p e) j -> p w (e j)", p=128, e=2)

    zbufs = 2
    assert n_ups <= 2, "PSUM budget: 4 rho + n_ups + 2 Z banks <= 8"
    with ExitStack() as ctx:
        tc = ctx.enter_context(tile.TileContext(nc))
        consts = ctx.enter_context(tc.tile_pool(name="consts", bufs=1))
        persist = ctx.enter_context(tc.tile_pool(name="persist", bufs=1))
        work = ctx.enter_context(tc.tile_pool(
            name="work", bufs=int(os.environ.get("KERNEL_WORK_BUFS", "2"))))
        psR = ctx.enter_context(tc.tile_pool(name="psR", bufs=1, space="PSUM"))
        psA = ctx.enter_context(
            tc.tile_pool(name="psA", bufs=max(2, n_ups), space="PSUM"))
        psZ = ctx.enter_context(
            tc.tile_pool(name="psZ", bufs=zbufs, space="PSUM"))

        # ---- constants (consolidated: 3 DMAs) ----
        cbfA = consts.tile([128, 64 + N_ITER * 128], bf16, tag="cbfA")
        cbfB = consts.tile([128, 512], bf16, tag="cbfB")
        cf32 = consts.tile([128, 129], f32, tag="cf32")
        wS = cbfA[:, 0:64]
        wZ = cbfA[:, 64:64 + N_ITER * 128]
        wG = cbfB[:, 0:128]
        wU = cbfB[:, 128:256]
        wO = cbfB[:, 256:384]
        wI10 = cbfB[:, 384:512]
        ident = cf32[:, 0:128]
        biasv = cf32[:, 128:129]

        # ---- persistent state (per-chunk tiles) ----
        x_nat = [persist.tile([128, 4, 128], f32, tag=f"xnat{d}",
                              name=f"xnat{d}") for d in range(8)]
        xb = [persist.tile([128, CH], bf16, tag=f"xb{d}", name=f"xb{d}")
              for d in range(8)]
        u_sb = [persist.tile([128, CH], f32, tag=f"usb{c}", name=f"usb{c}")
                for c in range(n_ups, 4)]
        ub = [persist.tile([128, CH], bf16, tag=f"ubc{c}", name=f"ubc{c}")
              for c in range(4)]
        osb = [persist.tile([128, 4, 128], f32, tag=f"osb{d}",
                            name=f"osb{d}") for d in range(8)]

        rho = [psR.tile([128, CH], f32, tag=f"rho{c}", name=f"rho{c}")
               for c in range(4)]

        # ---- DMAs: SP + ACT queues in parallel; ordered so chunk 0's
        # dependencies land first (the DMA device serializes transfers) ----
        nc.sync.dma_start(x_nat[0], xr[:, 0:4, :])
        nc.sync.dma_start(cf32, cf32_in[:, :])
        nc.sync.dma_start(x_nat[4], xr[:, 16:20, :])
        nc.sync.dma_start(cbfA, cbfA_in[:, :])
        nc.sync.dma_start(x_nat[1], xr[:, 4:8, :])
        nc.sync.dma_start(x_nat[5], xr[:, 20:24, :])
        nc.sync.dma_start(cbfB, cbfB_in[:, :])
        nc.sync.dma_start(x_nat[2], xr[:, 8:12, :])
        nc.sync.dma_start(x_nat[6], xr[:, 24:28, :])
        nc.sync.dma_start(x_nat[3], xr[:, 12:16, :])
        nc.sync.dma_start(x_nat[7], xr[:, 28:32, :])

        head_dve = set(
            int(v) for v in os.environ.get("KERNEL_HEAD_DVE", "3,6,7").split(",")
            if v != "")
        # ---- transpose to xT (bf16) + initial scores rho_0 ----
        for i, g in enumerate((0, 4, 1, 5, 2, 6, 3, 7)):
            pt = psA.tile([128, CH], f32, tag="T", name=f"pt{g}")
            for u4 in range(4):
                nc.tensor.transpose(pt[:, 128 * u4: 128 * (u4 + 1)],
                                    x_nat[g][:, u4, :], ident)
            if g in head_dve:
                nc.vector.tensor_copy(out=xb[g], in_=pt)
            else:
                nc.scalar.copy(xb[g], pt)
            if i % 2 == 1:
                c = i // 2
                nc.tensor.matmul(rho[c][0:64, :], wS, xb[c],
                                 start=True, stop=True)
                nc.tensor.matmul(rho[c][64:128, :], wS, xb[c + 4],
                                 start=True, stop=True)

        # u psum accumulators (tag-ring slots last used by the transposes)
        u_ps = [psA.tile([128, CH], f32, tag="T", name=f"u_ps{i}")
                for i in range(n_ups)]

        done_order = []
        pieces = []
        emitted = [0]

        def emit_tail_piece(c, h, fine=False):
            """Output path for chunk c, half h: po = pm^T u + m10 x0 (PE),
            then psum->sbuf copy and store.  fine=True splits the copy and
            store in half-tiles to shorten the final drain."""
            if True:
                po = psA.tile([128, CH], f32, tag="T", name=f"po{c}_{h}")
                for i4 in range(4):
                    posl = po[:, 128 * i4: 128 * (i4 + 1)]
                    nc.tensor.matmul(
                        posl,
                        ub[c][64 * h: 64 * (h + 1),
                              128 * i4: 128 * (i4 + 1)],
                        wO[64 * h: 64 * (h + 1), :],
                        start=True, stop=False)
                    d = 4 * h + c
                    nc.tensor.matmul(
                        posl, xb[d][:, 128 * i4: 128 * (i4 + 1)], wI10,
                        start=False, stop=True)
                d = 4 * h + c
                pov = po.rearrange("p (v j) -> p v j", v=4)
                if not fine:
                    if h == 0:
                        nc.scalar.copy(osb[d], pov)
                    else:
                        nc.vector.tensor_copy(out=osb[d], in_=pov)
                    nc.sync.dma_start(outr[:, 4 * d: 4 * (d + 1), :],
                                      osb[d])
                else:
                    for q2 in range(2):
                        vsl = slice(2 * q2, 2 * (q2 + 1))
                        if q2 == 0:
                            nc.scalar.copy(osb[d][:, vsl, :], pov[:, vsl, :])
                        else:
                            nc.vector.tensor_copy(out=osb[d][:, vsl, :],
                                                  in_=pov[:, vsl, :])
                        wsl = slice(4 * d + 2 * q2, 4 * d + 2 * (q2 + 1))
                        nc.sync.dma_start(outr[:, wsl, :],
                                          osb[d][:, vsl, :])

        # ---- skewed wavefront: chunk c runs iteration t at wave t+skew[c]
        stage_major = bool(int(os.environ.get("KERNEL_STAGE_MAJOR", "0")))
        for wave in range(N_ITER + max(skew)):
            active = [(c, wave - skew[c]) for c in range(4)
                      if 0 <= wave - skew[c] < N_ITER]
            stages = []
            for c, t in active:
                scale_t = float(m64 ** t)
                last = t == N_ITER - 1
                Ec = work.tile([128, CH], bf16, tag=f"E{c}",
                               name=f"E{t}_{c}")
                Vc = work.tile([128, CH], bf16, tag=f"V{c}",
                               name=f"V{t}_{c}")
                rzc = work.tile([128, CH], f32, tag=f"rz{c}",
                                name=f"rz{t}_{c}")

                def s1(c=c, t=t, Ec=Ec, scale_t=scale_t):
                    nc.scalar.activation(Ec, rho[c], func=Exp,
                                         bias=biasv, scale=scale_t)
                    pz = psZ.tile([128, CH], f32, tag="Z",
                                  name=f"pz{t}_{c}")
                    nc.tensor.matmul(pz, wZ[:, 128 * t: 128 * (t + 1)],
                                     Ec, start=True, stop=True)
                    return pz

                def s2(pz, c=c, t=t, Ec=Ec, Vc=Vc, rzc=rzc):
                    last = t == N_ITER - 1
                    nc.vector.reciprocal_approx_fast(out=rzc, in_=pz)
                    meng = nc.vector if c in mult_dve else nc.gpsimd
                    meng.tensor_mul(Vc, Ec, rzc)
                    if not last:
                        nc.tensor.matmul(rho[c], wG, Vc,
                                         start=False, stop=True,
                                         skip_group_check=True)
                    if c < n_ups:
                        nc.tensor.matmul(u_ps[c], wU, Vc,
                                         start=(t == 0), stop=last)
                    else:
                        usb = u_sb[c - n_ups]
                        if t == 0:
                            nc.gpsimd.tensor_copy(out=usb, in_=Vc)
                        else:
                            nc.gpsimd.tensor_add(usb, usb, Vc)
                    if last:
                        usrc = u_ps[c] if c < n_ups else u_sb[c - n_ups]
                        if c < n_ups:
                            nc.scalar.copy(ub[c], usrc)
                        else:
                            nc.vector.tensor_copy(out=ub[c], in_=usrc)
                        done_order.append(c)
                        pieces.extend([(c, 0), (c, 1)])
                        limit = (2 * len(done_order) -
                                 int(os.environ.get("KERNEL_TAIL_LAG", "1")))
                        while emitted[0] < min(limit, len(pieces)):
                            pc, ph = pieces[emitted[0]]
                            emit_tail_piece(pc, ph)
                            emitted[0] += 1
                stages.append((s1, s2))
            if stage_major:
                pzs = [f1() for f1, _ in stages]
                for (_, f2), pz in zip(stages, pzs):
                    f2(pz)
            else:
                for f1, f2 in stages:
                    f2(f1())
        fine = bool(int(os.environ.get("KERNEL_FINE_TAIL", "0")))
        while emitted[0] < len(pieces):
            pc, ph = pieces[emitted[0]]
            emit_tail_piece(pc, ph, fine=(emitted[0] == len(pieces) - 1
                                          and fine))
            emitted[0] += 1

    nc.finalize()
    return nc


def _host_constants(c, mu, A, alpha):
    """Host-side precompute.  Returns None unless P_k identical and
    M = I - alpha P is scalar (the shipped regime), else fall back."""
    c = np.asarray(c, np.float32)
    mu = np.asarray(mu, np.float32)
    A = np.asarray(A, np.float32)
    alpha = np.float32(alpha)
    P = np.einsum("kji,kjl->kil", A, A).astype(np.float32)
    if not np.allclose(P, P[0:1], rtol=1e-6, atol=1e-7):
        return None
    P0 = P[0].astype(np.float64)
    M = np.eye(D) - np.float64(alpha) * P0
    m0 = float(M[0, 0])
    if not np.allclose(M, m0 * np.eye(D), rtol=0, atol=1e-7):
        return None

    mu64 = mu.astype(np.float64)
    pm = mu64 @ P0.T                         # (K, D)
    bias = c.astype(np.float64) - 0.5 * np.einsum("kj,kj->k", mu64, pm)
    G = pm @ pm.T                            # (K, K)
    m64 = np.float64(m0)
    a64 = np.float64(alpha)

    import ml_dtypes
    bf = ml_dtypes.bfloat16
    pmf = pm.astype(np.float32)

    wS = np.zeros((128, 64), np.float32)
    wS[0:64, 0:32] = pmf.T
    wS[64:128, 32:64] = pmf.T

    # wZ_t = (1/c_t) * block-ones so rz = c_t/Z directly
    wZ = np.zeros((128, N_ITER * 128), np.float32)
    for t in range(N_ITER):
        ct = a64 * m64 ** (N_ITER - 1 - t)
        for g in range(4):
            wZ[32 * g: 32 * (g + 1),
               128 * t + 32 * g: 128 * t + 32 * (g + 1)] = 1.0 / ct

    # Gs = m^-10 G (block-diag x4)
    wG = np.zeros((128, 128), np.float32)
    Gs = (1.0 / m64 ** N_ITER) * G
    for g in range(4):
        wG[32 * g: 32 * (g + 1), 32 * g: 32 * (g + 1)] = Gs

    wU = np.eye(128, dtype=np.float32)

    wO = np.zeros((128, 128), np.float32)
    wO[0:32, 0:64] = pmf                     # even samples (e=0)
    wO[32:64, 64:128] = pmf                  # odd samples (e=1)
    wO[64:128] = wO[0:64]                    # replica for base-partition 64

    wI10 = float(m64 ** N_ITER) * np.eye(128, dtype=np.float32)

    ident = np.eye(128, dtype=np.float32)
    biasv = np.tile(bias.astype(np.float32), 4).reshape(128, 1)

    cbfA = np.concatenate([wS, wZ], axis=1).astype(bf)
    cbfB = np.concatenate([wG, wU, wO, wI10], axis=1).astype(bf)
    cf32 = np.concatenate([ident, biasv], axis=1).astype(np.float32)
    tensors = {"cbfA": cbfA, "cbfB": cbfB, "cf32": cf32}
    return tensors, m0, float(alpha)


def _numpy_fallback(x, c, mu, A, alpha):
    x = np.asarray(x, np.float32)
    c = np.asarray(c, np.float32)
    mu = np.asarray(mu, np.float32)
    A = np.asarray(A, np.float32)
    P = np.einsum("kji,kjl->kil", A, A).astype(np.float32)
    for _ in range(N_ITER):
        diff = x[:, None, :] - mu[None, :, :]
        Pd = np.einsum("kij,bkj->bki", P, diff)
        quad = np.einsum("bki,bki->bk", diff, Pd)
        s = c[None, :] - 0.5 * quad
        s = s - s.max(axis=1, keepdims=True)
        e = np.exp(s)
        w = e / e.sum(axis=1, keepdims=True)
        grad = -np.einsum("bk,bki->bi", w, Pd)
        x = x + np.float32(alpha) * grad
    return x.astype(np.float32)


def _cfg():
    n_ups = int(os.environ.get("KERNEL_N_UPS", "2"))
    mult_dve = tuple(
        int(v) for v in os.environ.get("KERNEL_MULT_DVE", "").split(",")
        if v != "")
    skew = tuple(
        int(v) for v in os.environ.get("KERNEL_SKEW", "0,1,1,1").split(","))
    return n_ups, mult_dve, skew


def kernel(x, c, mu, A, alpha):
    x = np.ascontiguousarray(np.asarray(x, np.float32))
    host = _host_constants(c, mu, A, alpha)
    if host is None:
        return _numpy_fallback(x, c, mu, A, alpha)
    consts, m0, a0 = host

    from concourse.bass_utils import run_bass_kernel_spmd

    cfg = (m0, a0) + _cfg()
    if _MODULE_CACHE.get("cfg") != cfg:
        _MODULE_CACHE["nc"] = _build_module(m0, a0, *_cfg())
        _MODULE_CACHE["cfg"] = cfg
    nc = _MODULE_CACHE["nc"]

    core_ids = list(range(N_CORES))
    in_maps = []
    for i in core_ids:
        mp = {"x": np.ascontiguousarray(x[i * BC: (i + 1) * BC])}
        mp.update(consts)
        in_maps.append(mp)

    trace = bool(int(os.environ.get("KERNEL_TRACE", "0")))
    res = run_bass_kernel_spmd(nc, in_maps, core_ids, trace=trace)
    kernel.last_results = res
    kernel.last_exec_time_ns = res.exec_time_ns
    outp = np.concatenate([res.results[i]["out"] for i in core_ids], axis=0)
    return outp.astype(np.float32)


kernel.last_exec_time_ns = None
kernel.last_results = None
